# revision 11
# baseline (speedup 1.0000x reference)
"""Trainium2 Bass kernel v4.2 for the dense_cnn problem:

    t1 = conv1x1(x, w1); t2 = gelu(t1)
    t5 = dwconv5x5(t2, w5, pad=2)
    t6 = dwconv7x7_dil3(t5, w6, pad=9)
    t7 = conv1x1(t6, w7); t8 = t7 * t2; t9 = conv1x1(t8, w9)
    out = x + t9

Sharding: data-parallel over batch N=32 across 8 cores (4 samples/core).

v4.2 design (profile + engine-rate LP):
  - Per-engine program order is execution order; the emission is
    software-pipelined across samples so the PE never head-of-line
    blocks on dw5: chain(n+1) runs on DVE while D(n) runs on PE.
  - dw7: all 25 fp8 DoubleRow pair-diagonal matmuls on PE; pair weights
    precomputed on the host and DMA'd.
  - dw5: 2 taps/group (dy=2, always row-in-bounds) on the PE as plain
    fp8-diag x f16 matmuls; their PSUM is ACT-copied into the f16
    accumulator as the chain init. The other 23 taps are mul+add chains:
    muls split across ACT/Pool/DVE, adds on DVE (tensor_tensor 2x mode).
    Vertical tap clipping instead of vertical padding.
  - t8 is never materialized: the E-stage multiply writes into t2pad
    in-place (dead after E) and the F matmuls read it via a strided rhs.
  - PSUM: every wave is a 4-bank tile alternating two tags -> wave i+1's
    matmuls always overlap wave i's eviction.
"""

import numpy as np
import ml_dtypes

import concourse.bass as bass
import concourse.mybir as mybir
from concourse.tile import TileContext
from concourse.bass_utils import run_bass_kernel_spmd

# ---------------------------------------------------------------------------
# Workaround: this walrus build rejects >N sem waits on the TileContext tail
# drain ("Too many sync wait commands"). Split them one-per-drain.
from concourse.vector_clock import ScopedClock, VectorClock


def _drain_and_barrier_split(self, tick_clock, wait_clock):
    vc = tick_clock.global_clock
    for proc in range(len(vc)):
        tick = vc[proc]
        if tick <= 0:
            continue
        d = self.nc.sync.drain()
        req = ScopedClock({None: VectorClock([0] * len(vc))})
        req.require_at_least(None, proc, tick)
        wait_clock.add_sem_waits(d.ins, req)
    self.nc.all_engine_barrier()
    assert self.sems is not None
    popped = self.nc._tile_sem_poison_stack.pop()
    assert popped is self._sem_poison
    self.nc.clear_and_free_semaphores(list(self.sems.allocated().values()))
    self.nc.all_engine_barrier()


TileContext._drain_and_barrier = _drain_and_barrier_split

# This walrus build also rejects >1 sem wait on regular engine instructions.
# Post-process the serialized BIR: hoist excess waits onto same-engine NoOps
# inserted right before the instruction.
import json as _json

_orig_to_json_bytes = bass.Bass.to_json_bytes


def _to_json_bytes_split_waits(self):
    d = _json.loads(_orig_to_json_bytes(self))
    ctr = 0
    for fn in d.get("functions", []):
        for blk in fn.get("blocks", []):
            insts = blk.get("instructions", [])
            out = []
            for inst in insts:
                si = inst.get("sync_info")
                waits = (si or {}).get("on_wait") or []
                if len(waits) > 1:
                    for w in waits[:-1]:
                        out.append({
                            "debug": inst.get("debug", 0),
                            "engine": inst["engine"],
                            "ins": [],
                            "outs": [],
                            "name": f"{inst['name']}_hw{ctr}",
                            "opcode": "NoOp",
                            "sync_info": {"on_wait": [w], "on_update": []},
                        })
                        ctr += 1
                    si["on_wait"] = waits[-1:]
                out.append(inst)
            blk["instructions"] = out
    return _json.dumps(d).encode()


bass.Bass.to_json_bytes = _to_json_bytes_split_waits
# ---------------------------------------------------------------------------

F16 = mybir.dt.float16
F32 = mybir.dt.float32
F8 = mybir.dt.float8e4
NP_F8 = ml_dtypes.float8_e4m3
AF = mybir.ActivationFunctionType
OP = mybir.AluOpType
PM = mybir.MatmulPerfMode

N_CORES = 8
NS = 4              # samples per core
C, H, W = 384, 56, 56
G = 3               # channel groups of 128
HW = H * W          # 3136
W5P = 60            # t2 padded row pitch (horizontal pad 2+2); rows NOT padded
W7P = 74            # t5 padded height (vertical pad 9+9)
W7PP = 80           # t5 row pitch (32B-aligned partition pitch for PE fp8)
CH_ROWS = 8         # output rows per PSUM chunk
BANK = 512          # f32 elems per PSUM bank
CHF = CH_ROWS * W   # 448 elems per chunk

# ---- dw5 tap split (tunables) --------------------------------------------
PE5_TAPS = [(2, 1), (2, 3)]      # on PE (dy=2: rows never clip)
DW5_LAST = (2, 4)                # final chain tap (full rows, writes f8 dst)
DW5_CHAIN = [
    (dy, dx)
    for dy in range(5)
    for dx in range(5)
    if (dy, dx) not in PE5_TAPS and (dy, dx) != DW5_LAST
] + [DW5_LAST]                   # 23 taps, mul+add chain
# mul engine per chain position (rotated per group for time-spread):
# per sample: 23 taps x 3 groups = 69 muls -> pool ~15, dve ~6, act ~48
_MUL_ENG = (["act", "act", "act", "pool", "dve"] * 5)[:23]

DW7_TAPS = [(jy, jx) for jy in range(7) for jx in range(7)]
DW7_PAIRS = [(DW7_TAPS[2 * i], DW7_TAPS[2 * i + 1]) for i in range(24)] + [
    (DW7_TAPS[48], None)
]


def _chunks_of(rows):
    out = []
    r = 0
    while r < rows:
        n = min(CH_ROWS, rows - r)
        out.append((r, n))
        r += n
    return out


def _waves_of(chunks):
    return [chunks[i : i + 4] for i in range(0, len(chunks), 4)]


ALL_CH = _chunks_of(H)          # 7 chunks of 8 rows
ALL_WAVES = _waves_of(ALL_CH)   # [4 chunks, 3 chunks]

import bass_rust as _br


class _Prog:
    def __init__(self):
        nc = bass.Bass("TRN2", target_bir_lowering=False, debug=False)
        self.nc = nc
        self.x_d = nc.dram_tensor("x16", [NS, G, 128, HW], F16, kind="ExternalInput")
        self.w1T_d = nc.dram_tensor("w1T", [G, 128, C], F16, kind="ExternalInput")
        self.w7T_d = nc.dram_tensor("w7T", [G, 128, C], F16, kind="ExternalInput")
        self.w9T_d = nc.dram_tensor("w9T", [G, 128, C], F16, kind="ExternalInput")
        self.w5t_d = nc.dram_tensor("w5t", [G, 128, 25], F32, kind="ExternalInput")
        self.dp6_d = nc.dram_tensor("dp6", [G, 25, 128, 256], F8, kind="ExternalInput")
        self.dp5_d = nc.dram_tensor(
            "dp5", [G, len(PE5_TAPS), 128, 128], F8, kind="ExternalInput"
        )
        self.o_d = nc.dram_tensor("out", [NS, G, 128, HW], F32, kind="ExternalOutput")
        self.psum_toggle = 0
        self.x16 = {}

    def psum_wave(self, tag_hint):
        t = self.psum_toggle
        self.psum_toggle ^= 1
        return self.pp.tile(
            [128, 4, BANK], F32, name=f"pw_{tag_hint}", tag=f"pw{t}", bufs=1
        )

    def emit_consts(self):
        nc, const = self.nc, self.const
        self.w1T = [const.tile([128, C], F16, name=f"w1T{k}") for k in range(G)]
        self.w7T = [const.tile([128, C], F16, name=f"w7T{k}") for k in range(G)]
        self.w9T = [const.tile([128, C], F16, name=f"w9T{k}") for k in range(G)]
        self.w5t = [const.tile([128, 25], F32, name=f"w5t{g}") for g in range(G)]
        self.dp6 = {}
        self.dp5 = {}
        for g in range(G):
            nc.sync.dma_start(out=self.w1T[g][:], in_=self.w1T_d.ap()[g])
            nc.sync.dma_start(out=self.w7T[g][:], in_=self.w7T_d.ap()[g])
            nc.sync.dma_start(out=self.w9T[g][:], in_=self.w9T_d.ap()[g])
            nc.sync.dma_start(out=self.w5t[g][:], in_=self.w5t_d.ap()[g])
            for pi in range(len(DW7_PAIRS)):
                t = const.tile([128, 2, 128], F8, name=f"dp6_{g}_{pi}")
                nc.sync.dma_start(
                    out=t.rearrange("p a b -> p (a b)")[:],
                    in_=self.dp6_d.ap()[g, pi],
                )
                self.dp6[(g, pi)] = t
            for ti in range(len(PE5_TAPS)):
                t = const.tile([128, 128], F8, name=f"dp5_{g}_{ti}")
                nc.sync.dma_start(out=t[:], in_=self.dp5_d.ap()[g, ti])
                self.dp5[(g, ti)] = t

    def emit_pads(self):
        nc = self.nc
        self.t2pad = [
            [self.pads.tile([128, H * W5P], F16, name=f"t2p{q}_{g}") for g in range(G)]
            for q in range(2)
        ]
        self.t5pad = [
            [self.pads.tile([128, W7P * W7PP], F8, name=f"t5p{q}_{g}") for g in range(G)]
            for q in range(2)
        ]
        for q in range(2):
            for g in range(G):
                nc.gpsimd.memset(self.t2pad[q][g][:], 0.0)
                nc.gpsimd.memset(self.t5pad[q][g][:], 0.0)
        self.t2p3 = [
            [t.rearrange("p (h w) -> p h w", w=W5P) for t in self.t2pad[q]]
            for q in range(2)
        ]
        self.t5p3 = [
            [t.rearrange("p (h w) -> p h w", w=W7PP) for t in self.t5pad[q]]
            for q in range(2)
        ]

    # -- stage A: load x (per-group tiles) --------------------------------
    def emit_load(self, n):
        xs = []
        for g in range(G):
            xl = self.xload.tile(
                [128, HW], F16, name=f"x_{n}_{g}", tag="xl", bufs=6
            )
            self.nc.sync.dma_start(out=xl[:], in_=self.x_d.ap()[n, g])
            xs.append(xl)
        self.x16[n] = xs

    # -- stage B: t1 = w1 @ x ; t2 = gelu(t1) -> t2pad interior -----------
    def emit_B(self, n):
        nc = self.nc
        q = n % 2
        for m in range(G):
            for wave in ALL_WAVES:
                pw = self.psum_wave(f"B{n}{m}")
                for k in range(G):
                    for ci, (r0, nr) in enumerate(wave):
                        nc.tensor.matmul(
                            pw[:, ci : ci + 1, 0 : nr * W],
                            self.w1T[k][:, 128 * m : 128 * (m + 1)],
                            self.x16[n][k][:, W * r0 : W * (r0 + nr)],
                            start=(k == 0),
                            stop=(k == G - 1),
                        )
                r0 = wave[0][0]
                rows = sum(nr for _, nr in wave)
                in_ap = pw[:, 0 : len(wave), 0 : CHF].rearrange(
                    "p c (r w) -> p c r w", w=W
                )
                out_ap = self.t2p3[q][m][:, r0 : r0 + rows, 2 : 2 + W].rearrange(
                    "p (c r) w -> p c r w", r=CH_ROWS
                )
                nc.scalar.activation(out_ap, in_ap, AF.Gelu)

    # -- stage B2: dw5 PE taps -> psum -> ACT copy into acc ---------------
    def emit_pe5(self, n):
        nc = self.nc
        q = n % 2
        self.acc = getattr(self, "acc", {})
        for g in range(G):
            a = self.dve.tile([128, HW], F16, name=f"acc{n}_{g}", tag="acc", bufs=3)
            self.acc[(n, g)] = a
            for wave in ALL_WAVES:
                pw = self.psum_wave(f"P5{n}{g}")
                for ti, (dy, dx) in enumerate(PE5_TAPS):
                    for ci, (r0, nr) in enumerate(wave):
                        # dy=2 taps: src rows r0..r0+nr always in bounds
                        nc.tensor.matmul(
                            pw[:, ci : ci + 1, 0 : nr * W],
                            self.dp5[(g, ti)][:],
                            self.t2p3[q][g][:, r0 : r0 + nr, dx : dx + W],
                            start=(ti == 0),
                            stop=(ti == len(PE5_TAPS) - 1),
                        )
                r0 = wave[0][0]
                nf = sum(nr for _, nr in wave) * W
                nc.scalar.activation(
                    a[:, W * r0 : W * r0 + nf].rearrange(
                        "p (c f) -> p c f", f=CHF
                    ),
                    pw[:, 0 : len(wave), 0 : CHF],
                    AF.Copy,
                )

    # -- stage C: dw5 chain taps (mul on ACT/Pool/DVE, add on DVE) --------
    def emit_dw5(self, n, lo=0, hi=None):
        nc = self.nc
        q = n % 2
        acc3 = [
            self.acc[(n, g)].rearrange("p (h w) -> p h w", w=W) for g in range(G)
        ]
        for ti, (dy, dx) in list(enumerate(DW5_CHAIN))[lo:hi]:
            last = ti == len(DW5_CHAIN) - 1
            for g in range(G):
                sc = self.w5t[g][:, 5 * dy + dx : 5 * dy + dx + 1]
                o0 = max(0, 2 - dy)
                o1 = min(H, H + 2 - dy)
                s0 = o0 + dy - 2
                src = self.t2p3[q][g][:, s0 : s0 + (o1 - o0), dx : dx + W]
                eng = _MUL_ENG[(ti + g) % len(_MUL_ENG)]
                tmp = self.dve.tile(
                    [128, HW], F16, name=f"tmp{n}_{ti}_{g}",
                    tag="tmp_a" if eng == "act" else "tmp_b", bufs=2,
                )
                tmp3 = tmp.rearrange("p (h w) -> p h w", w=W)[:, o0:o1, :]
                if eng == "act":
                    nc.scalar.activation(tmp3, src, AF.Copy, scale=sc)
                elif eng == "pool":
                    nc.gpsimd.tensor_scalar_mul(tmp3, src, sc)
                else:
                    nc.vector.tensor_scalar_mul(tmp3, src, sc)
                if last:
                    dst = self.t5p3[q][g][:, 9 : 9 + H, 9 : 9 + W]
                    nc.vector.tensor_tensor(dst, acc3[g][:], tmp3, OP.add)
                else:
                    a = acc3[g][:, o0:o1, :]
                    nc.vector.tensor_tensor(a, a, tmp3, OP.add)

    # -- stage D: t6 = dw7_dil3(t5) on PE (fp8 DoubleRow pairs) -----------
    def _pair_rhs(self, t5pad_g, pair, r0, nr):
        ta, tb = pair
        off_a = (r0 + 3 * ta[0]) * W7PP + 3 * ta[1]
        if tb is None:
            delta = -3  # in-bounds dummy read, zero diagonal kills it
        else:
            off_b = (r0 + 3 * tb[0]) * W7PP + 3 * tb[1]
            delta = off_b - off_a
        base = t5pad_g[:, off_a : off_a + 1]
        ap = base.copy()
        ap.ap = _br.VecI64Pair(
            [[W7P * W7PP, 128], [delta, 2], [W7PP, nr], [1, W]]
        )
        return ap

    def emit_D(self, n):
        nc = self.nc
        q = n % 2
        for g in range(G):
            t6g3 = self.t6[:, g * HW : (g + 1) * HW].rearrange(
                "p (h w) -> p h w", w=W
            )
            for wave in ALL_WAVES:
                pw = self.psum_wave(f"D{n}{g}")
                for pi in range(len(DW7_PAIRS)):
                    for ci, (r0, nr) in enumerate(wave):
                        nc.tensor.matmul(
                            pw[:, ci : ci + 1, 0 : nr * W],
                            self.dp6[(g, pi)][:],
                            self._pair_rhs(
                                self.t5pad[q][g], DW7_PAIRS[pi], r0, nr
                            ),
                            start=(pi == 0),
                            stop=(pi == len(DW7_PAIRS) - 1),
                            perf_mode=PM.DoubleRow,
                        )
                r0 = wave[0][0]
                rows = sum(nr for _, nr in wave)
                in_ap = pw[:, 0 : len(wave), 0 : CHF].rearrange(
                    "p c (r w) -> p c r w", w=W
                )
                out_ap = t6g3[:, r0 : r0 + rows, :].rearrange(
                    "p (c r) w -> p c r w", r=CH_ROWS
                )
                nc.scalar.activation(out_ap, in_ap, AF.Copy)

    # -- stage E: t7 = w7 @ t6 ; t8 = t7 * t2 in-place into t2pad ---------
    def emit_E(self, n):
        nc = self.nc
        q = n % 2
        for m in range(G):
            for wave in ALL_WAVES:
                pw = self.psum_wave(f"E{n}{m}")
                for k in range(G):
                    for ci, (r0, nr) in enumerate(wave):
                        nc.tensor.matmul(
                            pw[:, ci : ci + 1, 0 : nr * W],
                            self.w7T[k][:, 128 * m : 128 * (m + 1)],
                            self.t6[:, k * HW + W * r0 : k * HW + W * (r0 + nr)],
                            start=(k == 0),
                            stop=(k == G - 1),
                        )
                r0 = wave[0][0]
                rows = sum(nr for _, nr in wave)
                ps_ap = pw[:, 0 : len(wave), 0 : CHF].rearrange(
                    "p c (r w) -> p c r w", w=W
                )
                t2v = self.t2p3[q][m][:, r0 : r0 + rows, 2 : 2 + W].rearrange(
                    "p (c r) w -> p c r w", r=CH_ROWS
                )
                nc.vector.tensor_tensor(t2v, ps_ap, t2v, OP.mult)

    # -- stage F: t9 = w9 @ t8 ; out = x + t9 ; DMA out -------------------
    def emit_F(self, n):
        nc = self.nc
        q = n % 2
        for m in range(G):
            for wave in ALL_WAVES:
                pw = self.psum_wave(f"F{n}{m}")
                for k in range(G):
                    for ci, (r0, nr) in enumerate(wave):
                        nc.tensor.matmul(
                            pw[:, ci : ci + 1, 0 : nr * W],
                            self.w9T[k][:, 128 * m : 128 * (m + 1)],
                            self.t2p3[q][k][:, r0 : r0 + nr, 2 : 2 + W],
                            start=(k == 0),
                            stop=(k == G - 1),
                        )
                r0 = wave[0][0]
                nf = sum(nr for _, nr in wave) * W
                ost = self.small.tile(
                    [128, 4 * CHF], F32, name=f"os{n}{m}", tag="ost", bufs=1
                )
                ps_ap = pw[:, 0 : len(wave), 0 : CHF]
                nc.vector.tensor_tensor(
                    ost[:, 0:nf].rearrange("p (c f) -> p c f", f=CHF),
                    ps_ap,
                    self.x16[n][m][:, W * r0 : W * r0 + nf].rearrange(
                        "p (c f) -> p c f", f=CHF
                    ),
                    OP.add,
                )
                nc.sync.dma_start(
                    out=self.o_d.ap()[n, m, :, W * r0 : W * r0 + nf],
                    in_=ost[:, 0:nf],
                )


def _build_program():
    p = _Prog()
    nc = p.nc
    with TileContext(nc) as tc:
        with (
            tc.tile_pool(name="const", bufs=1) as p.const,
            tc.tile_pool(name="pads", bufs=1) as p.pads,
            tc.tile_pool(name="xload", bufs=2) as p.xload,
            tc.tile_pool(name="big", bufs=1) as p.big,
            tc.tile_pool(name="dve", bufs=1) as p.dve,
            tc.tile_pool(name="small", bufs=1) as p.small,
            tc.tile_pool(name="psum", bufs=2, space="PSUM") as p.pp,
        ):
            p.emit_consts()
            p.emit_pads()
            p.t6 = p.big.tile([128, G * HW], F16, name="t6")

            p.emit_load(0)
            p.emit_B(0)
            p.emit_pe5(0)
            p.emit_load(1)
            p.emit_B(1)
            p.emit_dw5(0)
            p.emit_pe5(1)
            SPLIT = 17
            for n in range(NS):
                p.emit_D(n)
                if n + 1 < NS:
                    p.emit_dw5(n + 1, 0, SPLIT)
                p.emit_E(n)
                if n + 1 < NS:
                    p.emit_dw5(n + 1, SPLIT, None)
                p.emit_F(n)
                if n + 2 < NS:
                    p.emit_load(n + 2)
                    p.emit_B(n + 2)
                    p.emit_pe5(n + 2)
    return nc


_NC_CACHE = None


def _get_nc():
    global _NC_CACHE
    if _NC_CACHE is None:
        _NC_CACHE = _build_program()
    return _NC_CACHE


def _prep_shared_inputs(w1, w5, w6, w7, w9):
    def lhsT(w):
        return (
            np.ascontiguousarray(np.asarray(w, np.float32).T)
            .astype(np.float16)
            .reshape(G, 128, C)
        )

    idx = np.arange(128)
    w6f = np.asarray(w6, np.float32).reshape(C, 49)
    dp6 = np.zeros((G, 25, 128, 256), NP_F8)
    for g in range(G):
        for pi, (ta, tb) in enumerate(DW7_PAIRS):
            blk = np.zeros((128, 256), np.float32)
            blk[idx, idx] = w6f[g * 128 : (g + 1) * 128, 7 * ta[0] + ta[1]]
            if tb is not None:
                blk[idx, 128 + idx] = w6f[g * 128 : (g + 1) * 128, 7 * tb[0] + tb[1]]
            dp6[g, pi] = blk.astype(NP_F8)

    w5f = np.asarray(w5, np.float32).reshape(C, 25)
    dp5 = np.zeros((G, len(PE5_TAPS), 128, 128), NP_F8)
    for g in range(G):
        for ti, (dy, dx) in enumerate(PE5_TAPS):
            blk = np.zeros((128, 128), np.float32)
            blk[idx, idx] = w5f[g * 128 : (g + 1) * 128, 5 * dy + dx]
            dp5[g, ti] = blk.astype(NP_F8)

    return {
        "w1T": lhsT(w1),
        "w7T": lhsT(w7),
        "w9T": lhsT(w9),
        "w5t": np.asarray(w5, np.float32).reshape(C, 25).reshape(G, 128, 25),
        "dp6": dp6,
        "dp5": dp5,
    }


def _make_in_maps(x, w1, w5, w6, w7, w9):
    x = np.asarray(x, np.float32)
    assert x.shape[0] == N_CORES * NS
    shared = _prep_shared_inputs(w1, w5, w6, w7, w9)
    x16 = x.astype(np.float16).reshape(N_CORES, NS, G, 128, HW)
    return [
        {"x16": np.ascontiguousarray(x16[i]), **shared} for i in range(N_CORES)
    ]


def kernel(x, w1, w5, w6, w7, w9, _trace=False, _tmpdir=None):
    in_maps = _make_in_maps(x, w1, w5, w6, w7, w9)
    nc = _get_nc()
    res = run_bass_kernel_spmd(
        nc, in_maps, core_ids=list(range(N_CORES)), trace=_trace, tmpdir=_tmpdir
    )
    outs = [res.results[i]["out"] for i in range(N_CORES)]
    out = np.stack(outs, axis=0).reshape(N_CORES * NS, C, H, W)
    if _trace:
        kernel.last_exec_time_ns = res.exec_time_ns
        kernel.last_results = res
    return out


# revision 12
# speedup vs baseline: 2.4748x; 2.4748x over previous
"""Trainium2 Bass kernel v4.2 for the dense_cnn problem:

    t1 = conv1x1(x, w1); t2 = gelu(t1)
    t5 = dwconv5x5(t2, w5, pad=2)
    t6 = dwconv7x7_dil3(t5, w6, pad=9)
    t7 = conv1x1(t6, w7); t8 = t7 * t2; t9 = conv1x1(t8, w9)
    out = x + t9

Sharding: data-parallel over batch N=32 across 8 cores (4 samples/core).

v4.2 design (profile + engine-rate LP):
  - Per-engine program order is execution order; the emission is
    software-pipelined across samples so the PE never head-of-line
    blocks on dw5: chain(n+1) runs on DVE while D(n) runs on PE.
  - dw7: all 25 fp8 DoubleRow pair-diagonal matmuls on PE; pair weights
    precomputed on the host and DMA'd.
  - dw5: 2 taps/group (dy=2, always row-in-bounds) on the PE as plain
    fp8-diag x f16 matmuls; their PSUM is ACT-copied into the f16
    accumulator as the chain init. The other 23 taps are mul+add chains:
    muls split across ACT/Pool/DVE, adds on DVE (tensor_tensor 2x mode).
    Vertical tap clipping instead of vertical padding.
  - t8 is never materialized: the E-stage multiply writes into t2pad
    in-place (dead after E) and the F matmuls read it via a strided rhs.
  - PSUM: every wave is a 4-bank tile alternating two tags -> wave i+1's
    matmuls always overlap wave i's eviction.
"""

import numpy as np
import ml_dtypes

import concourse.bass as bass
import concourse.mybir as mybir
from concourse.tile import TileContext
from concourse.bass_utils import run_bass_kernel_spmd

# ---------------------------------------------------------------------------
# Workaround: this walrus build rejects >N sem waits on the TileContext tail
# drain ("Too many sync wait commands"). Split them one-per-drain.
from concourse.vector_clock import ScopedClock, VectorClock


def _drain_and_barrier_split(self, tick_clock, wait_clock):
    vc = tick_clock.global_clock
    for proc in range(len(vc)):
        tick = vc[proc]
        if tick <= 0:
            continue
        d = self.nc.sync.drain()
        req = ScopedClock({None: VectorClock([0] * len(vc))})
        req.require_at_least(None, proc, tick)
        wait_clock.add_sem_waits(d.ins, req)
    self.nc.all_engine_barrier()
    assert self.sems is not None
    popped = self.nc._tile_sem_poison_stack.pop()
    assert popped is self._sem_poison
    self.nc.clear_and_free_semaphores(list(self.sems.allocated().values()))
    self.nc.all_engine_barrier()


TileContext._drain_and_barrier = _drain_and_barrier_split

# This walrus build also rejects >1 sem wait on regular engine instructions.
# Post-process the serialized BIR: hoist excess waits onto same-engine NoOps
# inserted right before the instruction.
import json as _json

_orig_to_json_bytes = bass.Bass.to_json_bytes


def _to_json_bytes_split_waits(self):
    d = _json.loads(_orig_to_json_bytes(self))
    ctr = 0
    for fn in d.get("functions", []):
        for blk in fn.get("blocks", []):
            insts = blk.get("instructions", [])
            out = []
            for inst in insts:
                si = inst.get("sync_info")
                waits = (si or {}).get("on_wait") or []
                if len(waits) > 1:
                    for w in waits[:-1]:
                        out.append({
                            "debug": inst.get("debug", 0),
                            "engine": inst["engine"],
                            "ins": [],
                            "outs": [],
                            "name": f"{inst['name']}_hw{ctr}",
                            "opcode": "NoOp",
                            "sync_info": {"on_wait": [w], "on_update": []},
                        })
                        ctr += 1
                    si["on_wait"] = waits[-1:]
                out.append(inst)
            blk["instructions"] = out
    return _json.dumps(d).encode()


bass.Bass.to_json_bytes = _to_json_bytes_split_waits
# ---------------------------------------------------------------------------

F16 = mybir.dt.float16
F32 = mybir.dt.float32
F8 = mybir.dt.float8e4
NP_F8 = ml_dtypes.float8_e4m3
AF = mybir.ActivationFunctionType
OP = mybir.AluOpType
PM = mybir.MatmulPerfMode

N_CORES = 8
NS = 4              # samples per core
C, H, W = 384, 56, 56
G = 3               # channel groups of 128
HW = H * W          # 3136
W5P = 60            # t2 padded row pitch (horizontal pad 2+2); rows NOT padded
W7P = 74            # t5 padded height (vertical pad 9+9)
W7PP = 80           # t5 row pitch (32B-aligned partition pitch for PE fp8)
CH_ROWS = 8         # output rows per PSUM chunk
BANK = 512          # f32 elems per PSUM bank
CHF = CH_ROWS * W   # 448 elems per chunk

# ---- dw5 tap split (tunables) --------------------------------------------
PE5_TAPS = [(2, 1), (2, 2), (2, 3)]  # on PE (dy=2: rows never clip)
DW5_LAST = (2, 4)                # final chain tap (full rows, writes f8 dst)
DW5_CHAIN = [
    (dy, dx)
    for dy in range(5)
    for dx in range(5)
    if (dy, dx) not in PE5_TAPS and (dy, dx) != DW5_LAST
] + [DW5_LAST]                   # 22 taps, mul+add chain
# mul engine per chain position (rotated per group for time-spread):
# per sample: 22 taps x 3 groups = 66 muls -> act ~53, dve ~13
_MUL_ENG = (["act", "act", "act", "act", "dve"] * 5)[:22]

DW7_TAPS = [(jy, jx) for jy in range(7) for jx in range(7)]
DW7_PAIRS = [(DW7_TAPS[2 * i], DW7_TAPS[2 * i + 1]) for i in range(24)] + [
    (DW7_TAPS[48], None)
]


def _chunks_of(rows):
    out = []
    r = 0
    while r < rows:
        n = min(CH_ROWS, rows - r)
        out.append((r, n))
        r += n
    return out


def _waves_of(chunks):
    return [chunks[i : i + 4] for i in range(0, len(chunks), 4)]


ALL_CH = _chunks_of(H)          # 7 chunks of 8 rows
ALL_WAVES = _waves_of(ALL_CH)   # [4 chunks, 3 chunks]

import bass_rust as _br


class _Prog:
    def __init__(self):
        nc = bass.Bass("TRN2", target_bir_lowering=False, debug=False)
        self.nc = nc
        self.x_d = nc.dram_tensor("x16", [NS, G, 128, HW], F16, kind="ExternalInput")
        self.w1T_d = nc.dram_tensor("w1T", [G, 128, C], F16, kind="ExternalInput")
        self.w7T_d = nc.dram_tensor("w7T", [G, 128, C], F16, kind="ExternalInput")
        self.w9T_d = nc.dram_tensor("w9T", [G, 128, C], F16, kind="ExternalInput")
        self.w5t_d = nc.dram_tensor("w5t", [G, 128, 25], F32, kind="ExternalInput")
        self.dp6_d = nc.dram_tensor("dp6", [G, 25, 128, 256], F8, kind="ExternalInput")
        self.dp5_d = nc.dram_tensor(
            "dp5", [G, len(PE5_TAPS), 128, 128], F8, kind="ExternalInput"
        )
        self.o_d = nc.dram_tensor("out", [NS, G, 128, HW], F32, kind="ExternalOutput")
        self.psum_toggle = 0
        self.x16 = {}

    def psum_wave(self, tag_hint):
        t = self.psum_toggle
        self.psum_toggle ^= 1
        return self.pp.tile(
            [128, 4, BANK], F32, name=f"pw_{tag_hint}", tag=f"pw{t}", bufs=1
        )

    def emit_consts(self):
        nc, const = self.nc, self.const
        self.w1T = [const.tile([128, C], F16, name=f"w1T{k}") for k in range(G)]
        self.w7T = [const.tile([128, C], F16, name=f"w7T{k}") for k in range(G)]
        self.w9T = [const.tile([128, C], F16, name=f"w9T{k}") for k in range(G)]
        self.w5t = [const.tile([128, 25], F32, name=f"w5t{g}") for g in range(G)]
        self.dp6 = {}
        self.dp5 = {}
        for g in range(G):
            nc.sync.dma_start(out=self.w1T[g][:], in_=self.w1T_d.ap()[g])
            nc.sync.dma_start(out=self.w7T[g][:], in_=self.w7T_d.ap()[g])
            nc.sync.dma_start(out=self.w9T[g][:], in_=self.w9T_d.ap()[g])
            nc.sync.dma_start(out=self.w5t[g][:], in_=self.w5t_d.ap()[g])
            for pi in range(len(DW7_PAIRS)):
                t = const.tile([128, 2, 128], F8, name=f"dp6_{g}_{pi}")
                nc.sync.dma_start(
                    out=t.rearrange("p a b -> p (a b)")[:],
                    in_=self.dp6_d.ap()[g, pi],
                )
                self.dp6[(g, pi)] = t
            for ti in range(len(PE5_TAPS)):
                t = const.tile([128, 128], F8, name=f"dp5_{g}_{ti}")
                nc.sync.dma_start(out=t[:], in_=self.dp5_d.ap()[g, ti])
                self.dp5[(g, ti)] = t

    def emit_pads(self):
        nc = self.nc
        self.t2pad = [
            [self.pads.tile([128, H * W5P], F16, name=f"t2p{q}_{g}") for g in range(G)]
            for q in range(2)
        ]
        self.t5pad = [
            [self.pads.tile([128, W7P * W7PP], F8, name=f"t5p{q}_{g}") for g in range(G)]
            for q in range(2)
        ]
        for q in range(2):
            for g in range(G):
                nc.gpsimd.memset(self.t2pad[q][g][:], 0.0)
                nc.gpsimd.memset(self.t5pad[q][g][:], 0.0)
        self.t2p3 = [
            [t.rearrange("p (h w) -> p h w", w=W5P) for t in self.t2pad[q]]
            for q in range(2)
        ]
        self.t5p3 = [
            [t.rearrange("p (h w) -> p h w", w=W7PP) for t in self.t5pad[q]]
            for q in range(2)
        ]

    # -- stage A: load x (per-group tiles) --------------------------------
    def emit_load(self, n):
        xs = []
        for g in range(G):
            xl = self.xload.tile(
                [128, HW], F16, name=f"x_{n}_{g}", tag="xl", bufs=6
            )
            self.nc.sync.dma_start(out=xl[:], in_=self.x_d.ap()[n, g])
            xs.append(xl)
        self.x16[n] = xs

    # -- stage B: t1 = w1 @ x ; t2 = gelu(t1) -> t2pad interior -----------
    def emit_B(self, n):
        nc = self.nc
        q = n % 2
        for m in range(G):
            for wave in ALL_WAVES:
                pw = self.psum_wave(f"B{n}{m}")
                for k in range(G):
                    for ci, (r0, nr) in enumerate(wave):
                        nc.tensor.matmul(
                            pw[:, ci : ci + 1, 0 : nr * W],
                            self.w1T[k][:, 128 * m : 128 * (m + 1)],
                            self.x16[n][k][:, W * r0 : W * (r0 + nr)],
                            start=(k == 0),
                            stop=(k == G - 1),
                        )
                r0 = wave[0][0]
                rows = sum(nr for _, nr in wave)
                in_ap = pw[:, 0 : len(wave), 0 : CHF].rearrange(
                    "p c (r w) -> p c r w", w=W
                )
                out_ap = self.t2p3[q][m][:, r0 : r0 + rows, 2 : 2 + W].rearrange(
                    "p (c r) w -> p c r w", r=CH_ROWS
                )
                nc.scalar.activation(out_ap, in_ap, AF.Gelu)

    # -- stage B2: dw5 PE taps -> psum -> ACT copy into acc ---------------
    def emit_pe5(self, n):
        nc = self.nc
        q = n % 2
        self.acc = getattr(self, "acc", {})
        for g in range(G):
            a = self.dve.tile([128, HW], F16, name=f"acc{n}_{g}", tag="acc", bufs=3)
            self.acc[(n, g)] = a
            for wave in ALL_WAVES:
                pw = self.psum_wave(f"P5{n}{g}")
                for ti, (dy, dx) in enumerate(PE5_TAPS):
                    for ci, (r0, nr) in enumerate(wave):
                        # dy=2 taps: src rows r0..r0+nr always in bounds
                        nc.tensor.matmul(
                            pw[:, ci : ci + 1, 0 : nr * W],
                            self.dp5[(g, ti)][:],
                            self.t2p3[q][g][:, r0 : r0 + nr, dx : dx + W],
                            start=(ti == 0),
                            stop=(ti == len(PE5_TAPS) - 1),
                        )
                r0 = wave[0][0]
                nf = sum(nr for _, nr in wave) * W
                nc.scalar.activation(
                    a[:, W * r0 : W * r0 + nf].rearrange(
                        "p (c f) -> p c f", f=CHF
                    ),
                    pw[:, 0 : len(wave), 0 : CHF],
                    AF.Copy,
                )

    # -- stage C: dw5 chain taps (mul on ACT/Pool/DVE, add on DVE) --------
    def emit_dw5(self, n, lo=0, hi=None):
        nc = self.nc
        q = n % 2
        acc3 = [
            self.acc[(n, g)].rearrange("p (h w) -> p h w", w=W) for g in range(G)
        ]
        for ti, (dy, dx) in list(enumerate(DW5_CHAIN))[lo:hi]:
            last = ti == len(DW5_CHAIN) - 1
            for g in range(G):
                sc = self.w5t[g][:, 5 * dy + dx : 5 * dy + dx + 1]
                o0 = max(0, 2 - dy)
                o1 = min(H, H + 2 - dy)
                s0 = o0 + dy - 2
                src = self.t2p3[q][g][:, s0 : s0 + (o1 - o0), dx : dx + W]
                eng = _MUL_ENG[(ti + g) % len(_MUL_ENG)]
                tmp = self.dve.tile(
                    [128, HW], F16, name=f"tmp{n}_{ti}_{g}",
                    tag="tmp_a" if eng == "act" else "tmp_b", bufs=2,
                )
                tmp3 = tmp.rearrange("p (h w) -> p h w", w=W)[:, o0:o1, :]
                if eng == "act":
                    nc.scalar.activation(tmp3, src, AF.Copy, scale=sc)
                elif eng == "pool":
                    nc.gpsimd.tensor_scalar_mul(tmp3, src, sc)
                else:
                    nc.vector.tensor_scalar_mul(tmp3, src, sc)
                if last:
                    dst = self.t5p3[q][g][:, 9 : 9 + H, 9 : 9 + W]
                    nc.vector.tensor_tensor(dst, acc3[g][:], tmp3, OP.add)
                else:
                    a = acc3[g][:, o0:o1, :]
                    nc.vector.tensor_tensor(a, a, tmp3, OP.add)

    # -- stage D: t6 = dw7_dil3(t5) on PE (fp8 DoubleRow pairs) -----------
    def _pair_rhs(self, t5pad_g, pair, r0, nr):
        ta, tb = pair
        off_a = (r0 + 3 * ta[0]) * W7PP + 3 * ta[1]
        if tb is None:
            delta = -3  # in-bounds dummy read, zero diagonal kills it
        else:
            off_b = (r0 + 3 * tb[0]) * W7PP + 3 * tb[1]
            delta = off_b - off_a
        base = t5pad_g[:, off_a : off_a + 1]
        ap = base.copy()
        ap.ap = _br.VecI64Pair(
            [[W7P * W7PP, 128], [delta, 2], [W7PP, nr], [1, W]]
        )
        return ap

    def emit_D(self, n):
        nc = self.nc
        q = n % 2
        for g in range(G):
            t6g3 = self.t6[:, g * HW : (g + 1) * HW].rearrange(
                "p (h w) -> p h w", w=W
            )
            for wave in ALL_WAVES:
                pw = self.psum_wave(f"D{n}{g}")
                for pi in range(len(DW7_PAIRS)):
                    for ci, (r0, nr) in enumerate(wave):
                        nc.tensor.matmul(
                            pw[:, ci : ci + 1, 0 : nr * W],
                            self.dp6[(g, pi)][:],
                            self._pair_rhs(
                                self.t5pad[q][g], DW7_PAIRS[pi], r0, nr
                            ),
                            start=(pi == 0),
                            stop=(pi == len(DW7_PAIRS) - 1),
                            perf_mode=PM.DoubleRow,
                        )
                r0 = wave[0][0]
                rows = sum(nr for _, nr in wave)
                in_ap = pw[:, 0 : len(wave), 0 : CHF].rearrange(
                    "p c (r w) -> p c r w", w=W
                )
                out_ap = t6g3[:, r0 : r0 + rows, :].rearrange(
                    "p (c r) w -> p c r w", r=CH_ROWS
                )
                nc.scalar.activation(out_ap, in_ap, AF.Copy)

    # -- stage E: t7 = w7 @ t6 ; t8 = t7 * t2 in-place into t2pad ---------
    def emit_E(self, n):
        nc = self.nc
        q = n % 2
        for m in range(G):
            for wave in ALL_WAVES:
                pw = self.psum_wave(f"E{n}{m}")
                for k in range(G):
                    for ci, (r0, nr) in enumerate(wave):
                        nc.tensor.matmul(
                            pw[:, ci : ci + 1, 0 : nr * W],
                            self.w7T[k][:, 128 * m : 128 * (m + 1)],
                            self.t6[:, k * HW + W * r0 : k * HW + W * (r0 + nr)],
                            start=(k == 0),
                            stop=(k == G - 1),
                        )
                r0 = wave[0][0]
                rows = sum(nr for _, nr in wave)
                ps_ap = pw[:, 0 : len(wave), 0 : CHF].rearrange(
                    "p c (r w) -> p c r w", w=W
                )
                t2v = self.t2p3[q][m][:, r0 : r0 + rows, 2 : 2 + W].rearrange(
                    "p (c r) w -> p c r w", r=CH_ROWS
                )
                nc.vector.tensor_tensor(t2v, ps_ap, t2v, OP.mult)

    # -- stage F: t9 = w9 @ t8 ; out = x + t9 ; DMA out -------------------
    def emit_F(self, n):
        nc = self.nc
        q = n % 2
        for m in range(G):
            for wave in ALL_WAVES:
                pw = self.psum_wave(f"F{n}{m}")
                for k in range(G):
                    for ci, (r0, nr) in enumerate(wave):
                        nc.tensor.matmul(
                            pw[:, ci : ci + 1, 0 : nr * W],
                            self.w9T[k][:, 128 * m : 128 * (m + 1)],
                            self.t2p3[q][k][:, r0 : r0 + nr, 2 : 2 + W],
                            start=(k == 0),
                            stop=(k == G - 1),
                        )
                r0 = wave[0][0]
                nf = sum(nr for _, nr in wave) * W
                ost = self.small.tile(
                    [128, 4 * CHF], F32, name=f"os{n}{m}", tag="ost", bufs=1
                )
                ps_ap = pw[:, 0 : len(wave), 0 : CHF]
                nc.vector.tensor_tensor(
                    ost[:, 0:nf].rearrange("p (c f) -> p c f", f=CHF),
                    ps_ap,
                    self.x16[n][m][:, W * r0 : W * r0 + nf].rearrange(
                        "p (c f) -> p c f", f=CHF
                    ),
                    OP.add,
                )
                nc.sync.dma_start(
                    out=self.o_d.ap()[n, m, :, W * r0 : W * r0 + nf],
                    in_=ost[:, 0:nf],
                )


def _build_program():
    p = _Prog()
    nc = p.nc
    with TileContext(nc) as tc:
        with (
            tc.tile_pool(name="const", bufs=1) as p.const,
            tc.tile_pool(name="pads", bufs=1) as p.pads,
            tc.tile_pool(name="xload", bufs=2) as p.xload,
            tc.tile_pool(name="big", bufs=1) as p.big,
            tc.tile_pool(name="dve", bufs=1) as p.dve,
            tc.tile_pool(name="small", bufs=1) as p.small,
            tc.tile_pool(name="psum", bufs=2, space="PSUM") as p.pp,
        ):
            p.emit_consts()
            p.emit_pads()
            p.t6 = p.big.tile([128, G * HW], F16, name="t6")

            p.emit_load(0)
            p.emit_B(0)
            p.emit_pe5(0)
            p.emit_load(1)
            p.emit_B(1)
            p.emit_dw5(0)
            p.emit_pe5(1)
            SPLIT = 17
            for n in range(NS):
                p.emit_D(n)
                if n + 1 < NS:
                    p.emit_dw5(n + 1, 0, SPLIT)
                p.emit_E(n)
                if n + 1 < NS:
                    p.emit_dw5(n + 1, SPLIT, None)
                p.emit_F(n)
                if n + 2 < NS:
                    p.emit_load(n + 2)
                    p.emit_B(n + 2)
                    p.emit_pe5(n + 2)
    return nc


_NC_CACHE = None


def _get_nc():
    global _NC_CACHE
    if _NC_CACHE is None:
        _NC_CACHE = _build_program()
    return _NC_CACHE


def _prep_shared_inputs(w1, w5, w6, w7, w9):
    def lhsT(w):
        return (
            np.ascontiguousarray(np.asarray(w, np.float32).T)
            .astype(np.float16)
            .reshape(G, 128, C)
        )

    idx = np.arange(128)
    w6f = np.asarray(w6, np.float32).reshape(C, 49)
    dp6 = np.zeros((G, 25, 128, 256), NP_F8)
    for g in range(G):
        for pi, (ta, tb) in enumerate(DW7_PAIRS):
            blk = np.zeros((128, 256), np.float32)
            blk[idx, idx] = w6f[g * 128 : (g + 1) * 128, 7 * ta[0] + ta[1]]
            if tb is not None:
                blk[idx, 128 + idx] = w6f[g * 128 : (g + 1) * 128, 7 * tb[0] + tb[1]]
            dp6[g, pi] = blk.astype(NP_F8)

    w5f = np.asarray(w5, np.float32).reshape(C, 25)
    dp5 = np.zeros((G, len(PE5_TAPS), 128, 128), NP_F8)
    for g in range(G):
        for ti, (dy, dx) in enumerate(PE5_TAPS):
            blk = np.zeros((128, 128), np.float32)
            blk[idx, idx] = w5f[g * 128 : (g + 1) * 128, 5 * dy + dx]
            dp5[g, ti] = blk.astype(NP_F8)

    return {
        "w1T": lhsT(w1),
        "w7T": lhsT(w7),
        "w9T": lhsT(w9),
        "w5t": np.asarray(w5, np.float32).reshape(C, 25).reshape(G, 128, 25),
        "dp6": dp6,
        "dp5": dp5,
    }


def _make_in_maps(x, w1, w5, w6, w7, w9):
    x = np.asarray(x, np.float32)
    assert x.shape[0] == N_CORES * NS
    shared = _prep_shared_inputs(w1, w5, w6, w7, w9)
    x16 = x.astype(np.float16).reshape(N_CORES, NS, G, 128, HW)
    return [
        {"x16": np.ascontiguousarray(x16[i]), **shared} for i in range(N_CORES)
    ]


def kernel(x, w1, w5, w6, w7, w9, _trace=False, _tmpdir=None):
    in_maps = _make_in_maps(x, w1, w5, w6, w7, w9)
    nc = _get_nc()
    res = run_bass_kernel_spmd(
        nc, in_maps, core_ids=list(range(N_CORES)), trace=_trace, tmpdir=_tmpdir
    )
    outs = [res.results[i]["out"] for i in range(N_CORES)]
    out = np.stack(outs, axis=0).reshape(N_CORES * NS, C, H, W)
    if _trace:
        kernel.last_exec_time_ns = res.exec_time_ns
        kernel.last_results = res
    return out
